# revision 18
# baseline (speedup 1.0000x reference)
"""Trainium2 Bass kernel for nn_DataEmbedding (linear embed + positional + GCN).

out[b,n,t,:] = x[b,n,t,:] @ W_lin + b_lin + pe[t,:] + gcn(emb_table)[n,:]

Sharding: graph-partitioned by destination node. Core k owns nodes
[625k, 625(k+1)) and produces the output shard out[:, 625k:625(k+1), :, :].
No collectives. Host does index/layout prep only (edge sort by destination,
padding, gather-map construction); all floating-point math runs on device.
"""

import math

import numpy as np

import concourse.bacc as bacc
import concourse.bass as bass
import concourse.mybir as mybir
from concourse.bass_utils import run_bass_kernel_spmd
from concourse.tile import TileContext

# problem constants (hardcoded per contract)
B, N, T, CIN, D, E = 8, 5000, 12, 3, 256, 160000
NCORES = 8
NPC = N // NCORES        # nodes per core = 625
BLK = 125                # destination nodes per PSUM block
NBLK = NPC // BLK        # blocks per core = 5
W = 4                    # gathered rows per index-column group
CHUNK = 128 * W          # edges per gather chunk = 512
NT = (N + 127) // 128    # global 128-node tiles = 40

f32 = mybir.dt.float32
i32 = mybir.dt.int32


def _pe_table() -> np.ndarray:
    pos = np.arange(T, dtype=np.float32)[:, None]
    div = np.exp(np.arange(0, D, 2, dtype=np.float32) * (-np.log(10000.0) / D))
    pe = np.zeros((T, D), dtype=np.float32)
    pe[:, 0::2] = np.sin(pos * div)
    pe[:, 1::2] = np.cos(pos * div)
    return pe


def _prep(x, edge_index, weights, W_lin, b_lin):
    """Host-side sharding/layout prep. Index manipulation and data movement
    only — no arithmetic on float input values."""
    ei = np.asarray(edge_index)
    row2 = np.concatenate([ei[0], np.arange(N)]).astype(np.int32)
    col2 = np.concatenate([ei[1], np.arange(N)]).astype(np.int32)
    w2 = np.concatenate(
        [np.asarray(weights, dtype=np.float32), np.ones(N, dtype=np.float32)]
    )
    order = np.argsort(col2, kind="stable")
    row_s, col_s, w_s = row2[order], col2[order], w2[order]

    # padded per-node weight matrix for on-device degree = row-sum
    starts = np.searchsorted(col_s, np.arange(N)).astype(np.int64)
    cnt = np.bincount(col2, minlength=N)
    L = int(max(8, ((cnt.max() + 7) // 8) * 8))
    NP = NT * 128  # 5120, node count padded to full 128-tiles
    wpad = np.zeros((NP, L), dtype=np.float32)
    offs = np.arange(len(col_s), dtype=np.int64) - starts[col_s]
    wpad[col_s, offs] = w_s
    wpad[N:, 0] = 1.0  # pad rows: deg=1 so dinv stays finite
    # partition-major packs: one DMA each on device
    wpad_pm = np.ascontiguousarray(
        wpad.reshape(NT, 128, L).transpose(1, 0, 2).reshape(128, NT * L)
    )

    # per-(core, block) sorted edge runs, padded to CHUNK multiples
    blk_lo = np.arange(NCORES * NBLK) * BLK
    seg_s = np.searchsorted(col_s, blk_lo)
    seg_e = np.searchsorted(col_s, blk_lo + BLK)
    CH = int(np.ceil((seg_e - seg_s).max() / CHUNK))

    # rhs rows on partitions 0..4, t along free dim: [5, T*D]
    pe = _pe_table()
    rhs60 = np.zeros((5, T * D), dtype=np.float32)
    for j in range(CIN):
        rhs60[j] = np.tile(np.asarray(W_lin, dtype=np.float32)[j], T)
    rhs60[3] = pe.reshape(-1)
    rhs60[4] = np.tile(np.asarray(b_lin, dtype=np.float32), T)

    xa = np.asarray(x, dtype=np.float32)
    per_core = []
    for k in range(NCORES):
        gidx = np.zeros((128, NBLK * CH * W), dtype=np.int32)
        wsv = np.zeros((128, NBLK * CH * W), dtype=np.float32)
        cloc = np.full((128, NBLK * CH * W), float(BLK), dtype=np.float32)
        for blk in range(NBLK):
            gi = NBLK * k + blk
            s, e = int(seg_s[gi]), int(seg_e[gi])
            n = e - s
            assert n <= CH * CHUNK
            pad = CH * CHUNK - n
            gi_p = np.pad(row_s[s:e], (0, pad))
            wv_p = np.pad(w_s[s:e], (0, pad))
            cl_p = np.pad(
                (col_s[s:e] - blk_lo[gi]).astype(np.float32),
                (0, pad),
                constant_values=float(BLK),
            )
            # slot (ch, j, p) -> column blk*CH*W + ch*W + j, partition p
            c0 = blk * CH * W
            gidx[:, c0 : c0 + CH * W] = (
                gi_p.reshape(CH, W, 128).transpose(2, 0, 1).reshape(128, CH * W)
            )
            wsv[:, c0 : c0 + CH * W] = (
                wv_p.reshape(CH, W, 128).transpose(2, 0, 1).reshape(128, CH * W)
            )
            cloc[:, c0 : c0 + CH * W] = (
                cl_p.reshape(CH, W, 128).transpose(2, 0, 1).reshape(128, CH * W)
            )

        # x in matmul-ready lhsT layout: [B, NBLK, 5, T*BLK] — K rows
        # {x_c0, x_c1, x_c2, 1, 1} on partitions 0..4, (t, node) along free
        xs = xa[:, k * NPC : (k + 1) * NPC].reshape(B, NBLK, BLK, T, CIN)
        x60 = np.ones((B, NBLK, 5, T, BLK), dtype=np.float32)
        x60[:, :, :CIN] = xs.transpose(0, 1, 4, 3, 2)
        per_core.append(
            {
                "gidx": gidx,
                "ws": wsv,
                "cloc": cloc,
                "x60": np.ascontiguousarray(x60.reshape(B, NBLK, 5, T * BLK)),
                "wpad_loc": np.ascontiguousarray(
                    wpad[k * NPC : (k + 1) * NPC]
                    .reshape(NBLK, BLK, L)
                    .transpose(1, 0, 2)
                    .reshape(BLK, NBLK * L)
                ),
            }
        )
    return per_core, wpad_pm, rhs60, CH, L


_KERNEL_CACHE: dict = {}


def _build_kernel(CH: int, L: int):
    key = (CH, L)
    if key in _KERNEL_CACHE:
        return _KERNEL_CACHE[key]

    nc = bacc.Bacc()
    x60_d = nc.declare_dram_parameter("x60", [B, NBLK, 5, T * BLK], f32, isOutput=False)
    gidx_d = nc.declare_dram_parameter("gidx", [128, NBLK * CH * W], i32, isOutput=False)
    ws_d = nc.declare_dram_parameter("ws", [128, NBLK * CH * W], f32, isOutput=False)
    cloc_d = nc.declare_dram_parameter("cloc", [128, NBLK * CH * W], f32, isOutput=False)
    wpad_d = nc.declare_dram_parameter("wpad", [128, NT * L], f32, isOutput=False)
    wploc_d = nc.declare_dram_parameter("wpad_loc", [BLK, NBLK * L], f32, isOutput=False)
    embT_d = nc.declare_dram_parameter("embT", [D, N], f32, isOutput=False)
    wg_d = nc.declare_dram_parameter("W_gcn", [D, D], f32, isOutput=False)
    bg_d = nc.declare_dram_parameter("b_gcn", [1, D], f32, isOutput=False)
    rhs60_d = nc.declare_dram_parameter("rhs60", [5, T * D], f32, isOutput=False)
    out_d = nc.declare_dram_parameter("out", [B, NPC, T, D], f32, isOutput=True)
    g_d = nc.dram_tensor("g_scaled", [NT * 128, D], f32)

    with TileContext(nc) as tc:
        with tc.tile_pool(name="const", bufs=1) as cp:
            # iota row pattern 0..BLK-1 repeated W times, as f32
            iota_i = cp.tile([128, W * BLK], i32)
            nc.gpsimd.iota(iota_i[:], pattern=[[0, W], [1, BLK]], base=0,
                           channel_multiplier=0)
            iota_f = cp.tile([128, W * BLK], f32)
            nc.vector.tensor_copy(iota_f[:], iota_i[:])
            ones_row = cp.tile([1, BLK], f32)
            nc.vector.memset(ones_row[:], 1.0)

            rhs60 = cp.tile([5, T * D], f32)
            nc.scalar.dma_start(out=rhs60[:], in_=rhs60_d[:])
            wg0 = cp.tile([128, D], f32)
            wg1 = cp.tile([128, D], f32)
            nc.scalar.dma_start(out=wg0[:], in_=wg_d[0:128, :])
            nc.scalar.dma_start(out=wg1[:], in_=wg_d[128:256, :])
            bg_row = cp.tile([1, D], f32)
            nc.scalar.dma_start(out=bg_row[:], in_=bg_d[:])

            gidx = cp.tile([128, NBLK * CH * W], i32)
            wsv = cp.tile([128, NBLK * CH * W], f32)
            cloc = cp.tile([128, NBLK * CH * W], f32)
            nc.scalar.dma_start(out=gidx[:], in_=gidx_d[:])
            nc.scalar.dma_start(out=wsv[:], in_=ws_d[:])
            nc.scalar.dma_start(out=cloc[:], in_=cloc_d[:])

            dinv_all = cp.tile([128, NT], f32)
            dinv_loc = cp.tile([BLK, NBLK], f32)
            ve_all = cp.tile([BLK, NBLK * D], f32)
            b_rep = cp.tile([BLK, D], f32)
            w_all = cp.tile([128, NT * L], f32)
            wl_all = cp.tile([BLK, NBLK * L], f32)
            eT0 = cp.tile([128, N], f32)
            eT1 = cp.tile([128, N], f32)
            g_all = cp.tile([128, NT * D], f32)
            nc.scalar.dma_start(out=w_all[:], in_=wpad_d[:])
            nc.scalar.dma_start(out=wl_all[:], in_=wploc_d[:])
            nc.scalar.dma_start(out=eT0[:], in_=embT_d[0:128, :])
            nc.scalar.dma_start(out=eT1[:], in_=embT_d[128:256, :])

            # ---- phase A: degrees -> dinv ----
            with (
                tc.tile_pool(name="pA", bufs=4) as pA,
                tc.tile_pool(name="ppA", bufs=2, space="PSUM") as ppA,
            ):
                for j in range(NT):
                    deg = pA.tile([128, 1], f32, tag="deg")
                    nc.vector.reduce_sum(
                        out=deg[:],
                        in_=w_all[:, j * L : (j + 1) * L],
                        axis=mybir.AxisListType.X,
                    )
                    rec = pA.tile([128, 1], f32, tag="rec")
                    nc.vector.reciprocal(rec[:], deg[:])
                    nc.scalar.sqrt(dinv_all[:, j : j + 1], rec[:])
                for blk in range(NBLK):
                    deg = pA.tile([BLK, 1], f32, tag="degl")
                    nc.vector.reduce_sum(
                        out=deg[:],
                        in_=wl_all[:, blk * L : (blk + 1) * L],
                        axis=mybir.AxisListType.X,
                    )
                    rec = pA.tile([BLK, 1], f32, tag="recl")
                    nc.vector.reciprocal(rec[:], deg[:])
                    nc.scalar.sqrt(dinv_loc[:, blk : blk + 1], rec[:])

                # ---- phase B: g = dinv * (emb @ W_gcn) -> one DRAM write ----
                nc.vector.memset(g_all[:, (NT - 1) * D :], 0.0)
                for j in range(NT):
                    cols = min(128, N - j * 128)
                    hg = ppA.tile([128, D], f32, space="PSUM", tag="hg")
                    nc.tensor.matmul(
                        hg[:cols, :],
                        lhsT=eT0[:, j * 128 : j * 128 + cols],
                        rhs=wg0[:],
                        start=True,
                        stop=False,
                    )
                    nc.tensor.matmul(
                        hg[:cols, :],
                        lhsT=eT1[:, j * 128 : j * 128 + cols],
                        rhs=wg1[:],
                        start=False,
                        stop=True,
                    )
                    nc.vector.tensor_scalar_mul(
                        g_all[:cols, j * D : (j + 1) * D],
                        hg[:cols, :],
                        dinv_all[:cols, j : j + 1],
                    )
                nc.sync.dma_start(
                    out=g_d[:].rearrange("(a p) d -> p a d", p=128),
                    in_=g_all[:].rearrange("p (a d) -> p a d", d=D),
                )
                # b_rep = ones(125,1) @ b_gcn(1,256)
                br = ppA.tile([BLK, D], f32, space="PSUM", tag="hg")
                nc.tensor.matmul(
                    br[:], lhsT=ones_row[0:1, :], rhs=bg_row[0:1, :], start=True, stop=True
                )
                nc.vector.tensor_copy(b_rep[:], br[:])

            # ---- phase C: per block, GCN scatter-matmul then main output ----
            with (
                tc.tile_pool(name="pC", bufs=4) as pC,
                tc.tile_pool(name="vps", bufs=2, space="PSUM") as vps,
                tc.tile_pool(name="mps", bufs=2, space="PSUM") as mps,
                tc.tile_pool(name="outp", bufs=3) as outp,
            ):
                for blk in range(NBLK):
                    vp = vps.tile([BLK, D], f32, space="PSUM", tag="vp")
                    for ch in range(CH):
                        c0 = (blk * CH + ch) * W
                        M = pC.tile([128, W * D], f32, tag="M")
                        # HW consumes one offset per partition per indirect DMA,
                        # so issue W single-column gathers (row j of each slot).
                        for j in range(W):
                            nc.gpsimd.indirect_dma_start(
                                out=M[:, j * D : (j + 1) * D],
                                out_offset=None,
                                in_=g_d[:],
                                in_offset=bass.IndirectOffsetOnAxis(
                                    ap=gidx[:, c0 + j : c0 + j + 1], axis=0
                                ),
                            )
                        # S = (iota == cloc) * ws ; cloc/ws broadcast along BLK
                        S0 = pC.tile([128, W * BLK], f32, tag="S")
                        nc.vector.tensor_tensor(
                            out=S0[:],
                            in0=iota_f[:],
                            in1=_bcast_cols(cloc, c0, W),
                            op=mybir.AluOpType.is_equal,
                        )
                        nc.vector.tensor_tensor(
                            out=S0[:],
                            in0=S0[:],
                            in1=_bcast_cols(wsv, c0, W),
                            op=mybir.AluOpType.mult,
                        )
                        for j in range(W):
                            nc.tensor.matmul(
                                vp[:],
                                lhsT=S0[:, j * BLK : (j + 1) * BLK],
                                rhs=M[:, j * D : (j + 1) * D],
                                start=(ch == 0 and j == 0),
                                stop=(ch == CH - 1 and j == W - 1),
                            )
                    ve = ve_all[:, blk * D : (blk + 1) * D]
                    nc.vector.tensor_scalar_mul(ve, vp[:], dinv_loc[:, blk : blk + 1])
                    nc.vector.tensor_add(ve, ve, b_rep[:])

                    for b in range(B):
                        x60t = pC.tile([5, T * BLK], f32, tag="x")
                        nc.scalar.dma_start(out=x60t[:], in_=x60_d[b, blk])
                        osb = outp.tile([BLK, T * D], f32, tag="osb")
                        for half in range(2):
                            mp = mps.tile([BLK, 6 * D], f32, space="PSUM", tag="mp")
                            for tt in range(6):
                                t = half * 6 + tt
                                nc.tensor.matmul(
                                    mp[:, tt * D : (tt + 1) * D],
                                    lhsT=x60t[:, t * BLK : (t + 1) * BLK],
                                    rhs=rhs60[:, t * D : (t + 1) * D],
                                    start=True,
                                    stop=True,
                                )
                            for tt in range(6):
                                t = half * 6 + tt
                                nc.vector.tensor_add(
                                    osb[:, t * D : (t + 1) * D],
                                    mp[:, tt * D : (tt + 1) * D],
                                    ve,
                                )
                        nc.sync.dma_start(
                            out=out_d[b, blk * BLK : (blk + 1) * BLK].rearrange(
                                "p t d -> p (t d)"
                            ),
                            in_=osb[:],
                        )

    nc.finalize()  # run bacc passes (reg alloc, TRN2 sync-wait splitting)
    _KERNEL_CACHE[key] = nc
    return nc


def _bcast_cols(tile, c0, w):
    """[128, w] column slice broadcast along the BLK free dim -> [128, w, BLK]."""
    return tile[:, c0 : c0 + w, None].to_broadcast([128, w, BLK])


LAST_RESULTS = None  # BassKernelResults of the most recent run (for profiling)


def kernel(x, x_mark, edge_index, weights, W_lin, b_lin, emb_table, W_gcn, b_gcn):
    global LAST_RESULTS
    per_core, wpad, rhs60, CH, L = _prep(x, edge_index, weights, W_lin, b_lin)
    nc = _build_kernel(CH, L)
    embT = np.ascontiguousarray(np.asarray(emb_table, dtype=np.float32).T)
    shared = {
        "wpad": wpad,
        "embT": embT,
        "W_gcn": np.asarray(W_gcn, dtype=np.float32),
        "b_gcn": np.asarray(b_gcn, dtype=np.float32).reshape(1, D),
        "rhs60": rhs60,
    }
    in_maps = [{**shared, **pc} for pc in per_core]
    res = run_bass_kernel_spmd(nc, in_maps, list(range(NCORES)))
    LAST_RESULTS = res
    shards = [res.results[k]["out"] for k in range(NCORES)]
    return np.concatenate(shards, axis=1)


# revision 20
# speedup vs baseline: 1.0198x; 1.0198x over previous
"""Trainium2 Bass kernel for nn_DataEmbedding (linear embed + positional + GCN).

out[b,n,t,:] = x[b,n,t,:] @ W_lin + b_lin + pe[t,:] + gcn(emb_table)[n,:]

Sharding: graph-partitioned by destination node. Core k owns nodes
[625k, 625(k+1)) and produces the output shard out[:, 625k:625(k+1), :, :].
No collectives. Host does index/layout prep only (edge sort by destination,
padding, gather-map construction); all floating-point math runs on device.

GCN message passing: g = D^-1/2 (emb @ W_gcn) is stored to DRAM in bf16;
per 128-edge chunk the source rows are gathered (indirect DMA, round-robin
over 4 SWDGE queues), a one-hot selection matrix S = (iota==dst_local)*w is
built on DVE, and the PE accumulates S^T @ M into PSUM per 125-node block.
"""

import numpy as np
import ml_dtypes

import concourse.bacc as bacc
import concourse.bass as bass
import concourse.mybir as mybir
from concourse.bass_utils import run_bass_kernel_spmd
from concourse.tile import TileContext

# problem constants (hardcoded per contract)
B, N, T, CIN, D, E = 8, 5000, 12, 3, 256, 160000
NCORES = 8
NPC = N // NCORES        # nodes per core = 625
BLK = 125                # destination nodes per PSUM block
NBLK = NPC // BLK        # blocks per core = 5
W = 4                    # 128-edge sub-chunks per S-build group
CHUNK = 128 * W          # edges per chunk group = 512
NT = (N + 127) // 128    # global 128-node tiles = 40
KX = 3 * T + 2           # main matmul contraction: (t,c) rows + pe + bias = 38

f32 = mybir.dt.float32
b16 = mybir.dt.bfloat16
i32 = mybir.dt.int32

_QUEUES = ["qPoolDynamic", "qPoolDynamic1", "qPoolDynamic2", "qPoolDynamic3"]


def _pe_table() -> np.ndarray:
    pos = np.arange(T, dtype=np.float32)[:, None]
    div = np.exp(np.arange(0, D, 2, dtype=np.float32) * (-np.log(10000.0) / D))
    pe = np.zeros((T, D), dtype=np.float32)
    pe[:, 0::2] = np.sin(pos * div)
    pe[:, 1::2] = np.cos(pos * div)
    return pe


def _prep(x, edge_index, weights, W_lin, b_lin):
    """Host-side sharding/layout prep. Index manipulation and data movement
    only — no arithmetic on float input values."""
    ei = np.asarray(edge_index)
    row2 = np.concatenate([ei[0], np.arange(N)]).astype(np.int32)
    col2 = np.concatenate([ei[1], np.arange(N)]).astype(np.int32)
    w2 = np.concatenate(
        [np.asarray(weights, dtype=np.float32), np.ones(N, dtype=np.float32)]
    )
    order = np.argsort(col2, kind="stable")
    row_s, col_s, w_s = row2[order], col2[order], w2[order]

    # padded per-node weight matrix for on-device degree = row-sum
    starts = np.searchsorted(col_s, np.arange(N)).astype(np.int64)
    cnt = np.bincount(col2, minlength=N)
    L = int(max(8, ((cnt.max() + 7) // 8) * 8))
    NP = NT * 128  # 5120, node count padded to full 128-tiles
    wpad = np.zeros((NP, L), dtype=np.float32)
    offs = np.arange(len(col_s), dtype=np.int64) - starts[col_s]
    wpad[col_s, offs] = w_s
    wpad[N:, 0] = 1.0  # pad rows: deg=1 so dinv stays finite
    wpad_pm = np.ascontiguousarray(
        wpad.reshape(NT, 128, L).transpose(1, 0, 2).reshape(128, NT * L)
    )

    # per-(core, block) sorted edge runs, padded to CHUNK multiples
    blk_lo = np.arange(NCORES * NBLK) * BLK
    seg_s = np.searchsorted(col_s, blk_lo)
    seg_e = np.searchsorted(col_s, blk_lo + BLK)
    CH = int(np.ceil((seg_e - seg_s).max() / CHUNK))

    # main-matmul rhs [KX, T*D]: rows 3t+c carry W_lin[c] in the t-block of
    # columns (block "diagonal"), row 36 = positional encoding, row 37 = b_lin
    pe = _pe_table()
    rhs38 = np.zeros((KX, T * D), dtype=np.float32)
    for t in range(T):
        for c in range(CIN):
            rhs38[3 * t + c, t * D : (t + 1) * D] = np.asarray(W_lin, np.float32)[c]
    rhs38[36] = pe.reshape(-1)
    rhs38[37] = np.tile(np.asarray(b_lin, dtype=np.float32), T)

    xa = np.asarray(x, dtype=np.float32)
    per_core = []
    for k in range(NCORES):
        gidx = np.zeros((128, NBLK * CH * W), dtype=np.int32)
        wsv = np.zeros((128, NBLK * CH * W), dtype=ml_dtypes.bfloat16)
        cloc = np.full((128, NBLK * CH * W), float(BLK), dtype=np.float32)
        for blk in range(NBLK):
            gi = NBLK * k + blk
            s, e = int(seg_s[gi]), int(seg_e[gi])
            n = e - s
            assert n <= CH * CHUNK
            pad = CH * CHUNK - n
            # g_d rows live in SBUF-native order: node n at row (n%128)*NT+n//128
            src = row_s[s:e].astype(np.int64)
            gi_p = np.pad((src % 128) * NT + src // 128, (0, pad))
            wv_p = np.pad(w_s[s:e], (0, pad))
            cl_p = np.pad(
                (col_s[s:e] - blk_lo[gi]).astype(np.float32),
                (0, pad),
                constant_values=float(BLK),
            )
            # slot (ch, j, p) -> column blk*CH*W + ch*W + j, partition p
            c0 = blk * CH * W
            gidx[:, c0 : c0 + CH * W] = (
                gi_p.reshape(CH, W, 128).transpose(2, 0, 1).reshape(128, CH * W)
            )
            wsv[:, c0 : c0 + CH * W] = (
                wv_p.reshape(CH, W, 128)
                .transpose(2, 0, 1)
                .reshape(128, CH * W)
                .astype(ml_dtypes.bfloat16)
            )
            cloc[:, c0 : c0 + CH * W] = (
                cl_p.reshape(CH, W, 128).transpose(2, 0, 1).reshape(128, CH * W)
            )

        # x in matmul-ready lhsT layout: [NBLK, KX, B*BLK] — K rows are
        # (t,c) pairs then two ones-rows (pe, bias); all 8 batches along free
        xs = xa[:, k * NPC : (k + 1) * NPC].reshape(B, NBLK, BLK, T, CIN)
        x38 = np.ones((NBLK, KX, B, BLK), dtype=np.float32)
        # [B, NBLK, BLK, T, CIN] -> [NBLK, T, CIN, B, BLK]
        x38[:, : 3 * T] = xs.transpose(1, 3, 4, 0, 2).reshape(NBLK, 3 * T, B, BLK)
        per_core.append(
            {
                "gidx": gidx,
                "ws": wsv,
                "cloc": cloc,
                "x38": np.ascontiguousarray(x38.reshape(NBLK, KX, B * BLK)),
                "wpad_loc": np.ascontiguousarray(
                    wpad[k * NPC : (k + 1) * NPC]
                    .reshape(NBLK, BLK, L)
                    .transpose(1, 0, 2)
                    .reshape(BLK, NBLK * L)
                ),
            }
        )
    return per_core, wpad_pm, rhs38, CH, L


_KERNEL_CACHE: dict = {}


def _build_kernel(CH: int, L: int):
    key = (CH, L)
    if key in _KERNEL_CACHE:
        return _KERNEL_CACHE[key]

    nc = bacc.Bacc(num_swdge_queues=4)
    x38_d = nc.declare_dram_parameter("x38", [NBLK, KX, B * BLK], f32, isOutput=False)
    gidx_d = nc.declare_dram_parameter("gidx", [128, NBLK * CH * W], i32, isOutput=False)
    ws_d = nc.declare_dram_parameter("ws", [128, NBLK * CH * W], b16, isOutput=False)
    cloc_d = nc.declare_dram_parameter("cloc", [128, NBLK * CH * W], f32, isOutput=False)
    wpad_d = nc.declare_dram_parameter("wpad", [128, NT * L], f32, isOutput=False)
    wploc_d = nc.declare_dram_parameter("wpad_loc", [BLK, NBLK * L], f32, isOutput=False)
    embT_d = nc.declare_dram_parameter("embT", [D, N], f32, isOutput=False)
    wg_d = nc.declare_dram_parameter("W_gcn", [D, D], f32, isOutput=False)
    bg_d = nc.declare_dram_parameter("b_gcn", [1, D], f32, isOutput=False)
    rhs38_d = nc.declare_dram_parameter("rhs38", [KX, T * D], f32, isOutput=False)
    out_d = nc.declare_dram_parameter("out", [B, NPC, T, D], f32, isOutput=True)
    g_d = nc.dram_tensor("g_scaled", [NT * 128, D], b16)

    def last_inst():
        return list(nc.inst_map.values())[-1]

    with TileContext(nc) as tc:
        with tc.tile_pool(name="const", bufs=1) as cp:
            # iota row pattern 0..BLK-1 repeated W times, as f32
            iota_i = cp.tile([128, W * BLK], i32)
            nc.gpsimd.iota(iota_i[:], pattern=[[0, W], [1, BLK]], base=0,
                           channel_multiplier=0)
            iota_f = cp.tile([128, W * BLK], f32)
            nc.vector.tensor_copy(iota_f[:], iota_i[:])
            ones_row = cp.tile([1, BLK], f32)
            nc.vector.memset(ones_row[:], 1.0)

            rhs38 = cp.tile([KX, T * D], f32)
            nc.scalar.dma_start(out=rhs38[:], in_=rhs38_d[:])
            wg0 = cp.tile([128, D], f32)
            wg1 = cp.tile([128, D], f32)
            nc.scalar.dma_start(out=wg0[:], in_=wg_d[0:128, :])
            nc.scalar.dma_start(out=wg1[:], in_=wg_d[128:256, :])
            bg_row = cp.tile([1, D], f32)
            nc.scalar.dma_start(out=bg_row[:], in_=bg_d[:])

            gidx = cp.tile([128, NBLK * CH * W], i32)
            wsv = cp.tile([128, NBLK * CH * W], b16)
            cloc = cp.tile([128, NBLK * CH * W], f32)
            nc.scalar.dma_start(out=gidx[:], in_=gidx_d[:])
            nc.scalar.dma_start(out=wsv[:], in_=ws_d[:])
            nc.scalar.dma_start(out=cloc[:], in_=cloc_d[:])

            dinv_all = cp.tile([128, NT], f32)
            dinv_loc = cp.tile([BLK, NBLK], f32)
            ve_all = cp.tile([BLK, NBLK * D], f32)
            b_rep = cp.tile([BLK, D], f32)
            w_all = cp.tile([128, NT * L], f32)
            wl_all = cp.tile([BLK, NBLK * L], f32)
            eT0 = cp.tile([128, N], f32)
            eT1 = cp.tile([128, N], f32)
            g_all = cp.tile([128, NT * D], b16)
            nc.scalar.dma_start(out=w_all[:], in_=wpad_d[:])
            nc.scalar.dma_start(out=wl_all[:], in_=wploc_d[:])
            nc.scalar.dma_start(out=eT0[:], in_=embT_d[0:128, :])
            nc.scalar.dma_start(out=eT1[:], in_=embT_d[128:256, :])

            # ---- phase A: degrees -> dinv ----
            with (
                tc.tile_pool(name="pA", bufs=4) as pA,
                tc.tile_pool(name="ppA", bufs=2, space="PSUM") as ppA,
            ):
                for j in range(NT):
                    deg = pA.tile([128, 1], f32, tag="deg")
                    nc.vector.reduce_sum(
                        out=deg[:],
                        in_=w_all[:, j * L : (j + 1) * L],
                        axis=mybir.AxisListType.X,
                    )
                    rec = pA.tile([128, 1], f32, tag="rec")
                    nc.vector.reciprocal(rec[:], deg[:])
                    nc.scalar.sqrt(dinv_all[:, j : j + 1], rec[:])
                for blk in range(NBLK):
                    deg = pA.tile([BLK, 1], f32, tag="degl")
                    nc.vector.reduce_sum(
                        out=deg[:],
                        in_=wl_all[:, blk * L : (blk + 1) * L],
                        axis=mybir.AxisListType.X,
                    )
                    rec = pA.tile([BLK, 1], f32, tag="recl")
                    nc.vector.reciprocal(rec[:], deg[:])
                    nc.scalar.sqrt(dinv_loc[:, blk : blk + 1], rec[:])

                # ---- phase B: g = dinv * (emb @ W_gcn) -> one DRAM write ----
                nc.vector.memset(g_all[:, (NT - 1) * D :], 0.0)
                for j in range(NT):
                    cols = min(128, N - j * 128)
                    hg = ppA.tile([128, D], f32, space="PSUM", tag="hg")
                    nc.tensor.matmul(
                        hg[:cols, :],
                        lhsT=eT0[:, j * 128 : j * 128 + cols],
                        rhs=wg0[:],
                        start=True,
                        stop=False,
                    )
                    nc.tensor.matmul(
                        hg[:cols, :],
                        lhsT=eT1[:, j * 128 : j * 128 + cols],
                        rhs=wg1[:],
                        start=False,
                        stop=True,
                    )
                    nc.vector.tensor_scalar_mul(
                        g_all[:cols, j * D : (j + 1) * D],
                        hg[:cols, :],
                        dinv_all[:cols, j : j + 1],
                    )
                nc.sync.dma_start(out=g_d[:].rearrange("(p a) d -> p a d", p=128),
                                  in_=g_all[:].rearrange("p (a d) -> p a d", d=D))
                # b_rep = ones(125,1) @ b_gcn(1,256)
                br = ppA.tile([BLK, D], f32, space="PSUM", tag="hg")
                nc.tensor.matmul(
                    br[:], lhsT=ones_row[0:1, :], rhs=bg_row[0:1, :], start=True, stop=True
                )
                nc.vector.tensor_copy(b_rep[:], br[:])

            # ---- phase C: per block, GCN scatter-matmul then main output ----
            with (
                tc.tile_pool(name="pC", bufs=4) as pC,
                tc.tile_pool(name="xload", bufs=2) as xload,
                tc.tile_pool(name="vps", bufs=2, space="PSUM") as vps,
                tc.tile_pool(name="mps", bufs=2, space="PSUM") as mps,
                tc.tile_pool(name="outp", bufs=3) as outp,
            ):
                qi = 0
                for blk in range(NBLK):
                    vp = vps.tile([BLK, D], f32, space="PSUM", tag="vp")
                    for ch in range(CH):
                        c0 = (blk * CH + ch) * W
                        M = pC.tile([128, W * D], b16, tag="M")
                        for j in range(W):
                            nc.gpsimd.indirect_dma_start(
                                out=M[:, j * D : (j + 1) * D],
                                out_offset=None,
                                in_=g_d[:],
                                in_offset=bass.IndirectOffsetOnAxis(
                                    ap=gidx[:, c0 + j : c0 + j + 1], axis=0
                                ),
                            )
                            last_inst().queue = _QUEUES[qi % 4]
                            qi += 1
                        # S = (iota == cloc) * ws ; cloc/ws broadcast along BLK
                        S0 = pC.tile([128, W * BLK], b16, tag="S")
                        nc.vector.tensor_tensor(
                            out=S0[:],
                            in0=iota_f[:],
                            in1=cloc[:, c0 : c0 + W, None].to_broadcast(
                                [128, W, BLK]
                            ),
                            op=mybir.AluOpType.is_equal,
                        )
                        nc.vector.tensor_tensor(
                            out=S0[:],
                            in0=S0[:],
                            in1=wsv[:, c0 : c0 + W, None].to_broadcast([128, W, BLK]),
                            op=mybir.AluOpType.mult,
                        )
                        for j in range(W):
                            nc.tensor.matmul(
                                vp[:],
                                lhsT=S0[:, j * BLK : (j + 1) * BLK],
                                rhs=M[:, j * D : (j + 1) * D],
                                start=(ch == 0 and j == 0),
                                stop=(ch == CH - 1 and j == W - 1),
                            )
                    ve = ve_all[:, blk * D : (blk + 1) * D]
                    nc.vector.tensor_scalar_mul(ve, vp[:], dinv_loc[:, blk : blk + 1])
                    nc.vector.tensor_add(ve, ve, b_rep[:])
                    ve3 = ve.rearrange("p d -> p () d").to_broadcast([BLK, T // 2, D])

                    x38t = xload.tile([KX, B * BLK], f32, tag="x")
                    nc.scalar.dma_start(out=x38t[:], in_=x38_d[blk])
                    for b in range(B):
                        lhsT = x38t[:, b * BLK : (b + 1) * BLK]
                        osb = outp.tile([BLK, T * D], f32, tag="osb")
                        for half in range(2):
                            mp = mps.tile([BLK, 3 * 512], f32, space="PSUM", tag="mp")
                            for i in range(3):
                                tp = half * 3 + i  # t-pair index
                                nc.tensor.matmul(
                                    mp[:, i * 512 : (i + 1) * 512],
                                    lhsT=lhsT,
                                    rhs=rhs38[:, tp * 512 : (tp + 1) * 512],
                                    start=True,
                                    stop=True,
                                )
                            nc.vector.tensor_tensor(
                                out=osb[:, half * 1536 : (half + 1) * 1536].rearrange(
                                    "p (t d) -> p t d", d=D
                                ),
                                in0=mp[:].rearrange("p (t d) -> p t d", d=D),
                                in1=ve3,
                                op=mybir.AluOpType.add,
                            )
                        nc.sync.dma_start(
                            out=out_d[b, blk * BLK : (blk + 1) * BLK].rearrange(
                                "p t d -> p (t d)"
                            ),
                            in_=osb[:],
                        )

    nc.finalize()  # run bacc passes (reg alloc, TRN2 sync-wait splitting)
    _KERNEL_CACHE[key] = nc
    return nc


LAST_RESULTS = None  # BassKernelResults of the most recent run (for profiling)


def kernel(x, x_mark, edge_index, weights, W_lin, b_lin, emb_table, W_gcn, b_gcn):
    global LAST_RESULTS
    per_core, wpad, rhs38, CH, L = _prep(x, edge_index, weights, W_lin, b_lin)
    nc = _build_kernel(CH, L)
    embT = np.ascontiguousarray(np.asarray(emb_table, dtype=np.float32).T)
    shared = {
        "wpad": wpad,
        "embT": embT,
        "W_gcn": np.asarray(W_gcn, dtype=np.float32),
        "b_gcn": np.asarray(b_gcn, dtype=np.float32).reshape(1, D),
        "rhs38": rhs38,
    }
    in_maps = [{**shared, **pc} for pc in per_core]
    res = run_bass_kernel_spmd(nc, in_maps, list(range(NCORES)))
    LAST_RESULTS = res
    shards = [res.results[k]["out"] for k in range(NCORES)]
    return np.concatenate(shards, axis=1)


# revision 25
# speedup vs baseline: 1.0528x; 1.0323x over previous
"""Trainium2 Bass kernel for nn_DataEmbedding (linear embed + positional + GCN).

out[b,n,t,:] = x[b,n,t,:] @ W_lin + b_lin + pe[t,:] + gcn(emb_table)[n,:]

Sharding: graph-partitioned by destination node. Core k owns nodes
[625k, 625(k+1)) and produces the output shard out[:, 625k:625(k+1), :, :].
No collectives. Host does index/layout prep only (edge sort by destination,
padding, gather-map construction); all floating-point math runs on device.

GCN message passing: g = D^-1/2 (emb @ W_gcn) is stored to DRAM in bf16;
per 128-edge chunk the source rows are gathered (indirect DMA, round-robin
over 4 SWDGE queues), a one-hot selection matrix S = (iota==dst_local)*w is
built on DVE, and the PE accumulates S^T @ M into PSUM per 125-node block.
"""

import numpy as np
import ml_dtypes

import concourse.bacc as bacc
import concourse.bass as bass
import concourse.mybir as mybir
from concourse.bass_utils import run_bass_kernel_spmd
from concourse.tile import TileContext

# problem constants (hardcoded per contract)
B, N, T, CIN, D, E = 8, 5000, 12, 3, 256, 160000
NCORES = 8
NPC = N // NCORES        # nodes per core = 625
BLK = 125                # destination nodes per PSUM block
NBLK = NPC // BLK        # blocks per core = 5
W = 4                    # 128-edge sub-chunks per S-build group
CHUNK = 128 * W          # edges per chunk group = 512
NT = (N + 127) // 128    # global 128-node tiles = 40
KX = 3 * T + 2           # main matmul contraction: (t,c) rows + pe + bias = 38

f32 = mybir.dt.float32
b16 = mybir.dt.bfloat16
i32 = mybir.dt.int32

_QUEUES = ["qPoolDynamic", "qPoolDynamic1", "qPoolDynamic2", "qPoolDynamic3"]


def _pe_table() -> np.ndarray:
    pos = np.arange(T, dtype=np.float32)[:, None]
    div = np.exp(np.arange(0, D, 2, dtype=np.float32) * (-np.log(10000.0) / D))
    pe = np.zeros((T, D), dtype=np.float32)
    pe[:, 0::2] = np.sin(pos * div)
    pe[:, 1::2] = np.cos(pos * div)
    return pe


def _prep(x, edge_index, weights, W_lin, b_lin):
    """Host-side sharding/layout prep. Index manipulation and data movement
    only — no arithmetic on float input values."""
    ei = np.asarray(edge_index)
    row2 = np.concatenate([ei[0], np.arange(N)]).astype(np.int32)
    col2 = np.concatenate([ei[1], np.arange(N)]).astype(np.int32)
    w2 = np.concatenate(
        [np.asarray(weights, dtype=np.float32), np.ones(N, dtype=np.float32)]
    )
    order = np.argsort(col2, kind="stable")
    row_s, col_s, w_s = row2[order], col2[order], w2[order]

    # padded per-node weight matrix for on-device degree = row-sum
    starts = np.searchsorted(col_s, np.arange(N)).astype(np.int64)
    cnt = np.bincount(col2, minlength=N)
    L = int(max(8, ((cnt.max() + 7) // 8) * 8))
    NP = NT * 128  # 5120, node count padded to full 128-tiles
    wpad = np.zeros((NP, L), dtype=np.float32)
    offs = np.arange(len(col_s), dtype=np.int64) - starts[col_s]
    wpad[col_s, offs] = w_s
    wpad[N:, 0] = 1.0  # pad rows: deg=1 so dinv stays finite
    wpad_pm = np.ascontiguousarray(
        wpad.reshape(NT, 128, L).transpose(1, 0, 2).reshape(128, NT * L)
    )

    # per-(core, block) sorted edge runs, padded to CHUNK multiples
    blk_lo = np.arange(NCORES * NBLK) * BLK
    seg_s = np.searchsorted(col_s, blk_lo)
    seg_e = np.searchsorted(col_s, blk_lo + BLK)
    CH = int(np.ceil((seg_e - seg_s).max() / CHUNK))

    # main-matmul rhs [KX, T*D]: rows 3t+c carry W_lin[c] in the t-block of
    # columns (block "diagonal"), row 36 = positional encoding, row 37 = b_lin
    pe = _pe_table()
    rhs38 = np.zeros((KX, T * D), dtype=np.float32)
    for t in range(T):
        for c in range(CIN):
            rhs38[3 * t + c, t * D : (t + 1) * D] = np.asarray(W_lin, np.float32)[c]
    rhs38[36] = pe.reshape(-1)
    rhs38[37] = np.tile(np.asarray(b_lin, dtype=np.float32), T)

    xa = np.asarray(x, dtype=np.float32)
    SW = CHUNK // 16  # int16 idx columns per chunk (wrapped in 16 partitions)
    per_core = []
    for k in range(NCORES):
        gidx = np.zeros((16, NBLK * CH * SW), dtype=np.int16)
        wsv = np.zeros((128, NBLK * CH * W), dtype=ml_dtypes.bfloat16)
        cloc = np.full((128, NBLK * CH * W), float(BLK), dtype=np.float32)
        for blk in range(NBLK):
            gi = NBLK * k + blk
            s, e = int(seg_s[gi]), int(seg_e[gi])
            n = e - s
            assert n <= CH * CHUNK
            pad = CH * CHUNK - n
            # g_d rows live in SBUF-native order: node n at row (n%128)*NT+n//128
            src = row_s[s:e].astype(np.int64)
            gi_p = np.pad((src % 128) * NT + src // 128, (0, pad))
            wv_p = np.pad(w_s[s:e], (0, pad))
            cl_p = np.pad(
                (col_s[s:e] - blk_lo[gi]).astype(np.float32),
                (0, pad),
                constant_values=float(BLK),
            )
            # dma_gather idx wrap: chunk slot i=(j*128+p) at [i%16, i//16]
            c0i = blk * CH * SW
            gidx[:, c0i : c0i + CH * SW] = (
                gi_p.reshape(CH, SW, 16).transpose(2, 0, 1).reshape(16, CH * SW)
            )
            # S-build slot (ch, j, p) -> column blk*CH*W + ch*W + j, partition p
            c0 = blk * CH * W
            wsv[:, c0 : c0 + CH * W] = (
                wv_p.reshape(CH, W, 128)
                .transpose(2, 0, 1)
                .reshape(128, CH * W)
                .astype(ml_dtypes.bfloat16)
            )
            cloc[:, c0 : c0 + CH * W] = (
                cl_p.reshape(CH, W, 128).transpose(2, 0, 1).reshape(128, CH * W)
            )

        # x in matmul-ready lhsT layout: [NBLK, KX, B*BLK] — K rows are
        # (t,c) pairs then two ones-rows (pe, bias); all 8 batches along free
        xs = xa[:, k * NPC : (k + 1) * NPC].reshape(B, NBLK, BLK, T, CIN)
        x38 = np.ones((NBLK, KX, B, BLK), dtype=np.float32)
        # [B, NBLK, BLK, T, CIN] -> [NBLK, T, CIN, B, BLK]
        x38[:, : 3 * T] = xs.transpose(1, 3, 4, 0, 2).reshape(NBLK, 3 * T, B, BLK)
        per_core.append(
            {
                "gidx": np.ascontiguousarray(np.tile(gidx, (8, 1))),
                "ws": wsv,
                "cloc": cloc,
                "x38": np.ascontiguousarray(x38.reshape(NBLK, KX, B * BLK)),
                "wpad_loc": np.ascontiguousarray(
                    wpad[k * NPC : (k + 1) * NPC]
                    .reshape(NBLK, BLK, L)
                    .transpose(1, 0, 2)
                    .reshape(BLK, NBLK * L)
                ),
            }
        )
    return per_core, wpad_pm, rhs38, CH, L


_KERNEL_CACHE: dict = {}


def _build_kernel(CH: int, L: int):
    key = (CH, L)
    if key in _KERNEL_CACHE:
        return _KERNEL_CACHE[key]

    nc = bacc.Bacc(num_swdge_queues=4)
    x38_d = nc.declare_dram_parameter("x38", [NBLK, KX, B * BLK], f32, isOutput=False)
    SW = CHUNK // 16
    gidx_d = nc.declare_dram_parameter(
        "gidx", [128, NBLK * CH * SW], mybir.dt.int16, isOutput=False
    )
    ws_d = nc.declare_dram_parameter("ws", [128, NBLK * CH * W], b16, isOutput=False)
    cloc_d = nc.declare_dram_parameter("cloc", [128, NBLK * CH * W], f32, isOutput=False)
    wpad_d = nc.declare_dram_parameter("wpad", [128, NT * L], f32, isOutput=False)
    wploc_d = nc.declare_dram_parameter("wpad_loc", [BLK, NBLK * L], f32, isOutput=False)
    embT_d = nc.declare_dram_parameter("embT", [D, N], f32, isOutput=False)
    wg_d = nc.declare_dram_parameter("W_gcn", [D, D], f32, isOutput=False)
    bg_d = nc.declare_dram_parameter("b_gcn", [1, D], f32, isOutput=False)
    rhs38_d = nc.declare_dram_parameter("rhs38", [KX, T * D], f32, isOutput=False)
    out_d = nc.declare_dram_parameter("out", [B, NPC, T, D], f32, isOutput=True)
    g_d = nc.dram_tensor("g_scaled", [NT * 128, D], b16)

    def last_inst():
        return list(nc.inst_map.values())[-1]

    with TileContext(nc) as tc:
        with tc.tile_pool(name="const", bufs=1) as cp:
            # iota row pattern 0..BLK-1 repeated W times, as f32
            iota_i = cp.tile([128, W * BLK], i32)
            nc.gpsimd.iota(iota_i[:], pattern=[[0, W], [1, BLK]], base=0,
                           channel_multiplier=0)
            iota_f = cp.tile([128, W * BLK], f32)
            nc.vector.tensor_copy(iota_f[:], iota_i[:])
            ones_row = cp.tile([1, BLK], f32)
            nc.vector.memset(ones_row[:], 1.0)

            rhs38 = cp.tile([KX, T * D], f32)
            nc.scalar.dma_start(out=rhs38[:], in_=rhs38_d[:])
            wg0 = cp.tile([128, D], f32)
            wg1 = cp.tile([128, D], f32)
            nc.scalar.dma_start(out=wg0[:], in_=wg_d[0:128, :])
            nc.scalar.dma_start(out=wg1[:], in_=wg_d[128:256, :])
            bg_row = cp.tile([1, D], f32)
            nc.scalar.dma_start(out=bg_row[:], in_=bg_d[:])

            gidx = cp.tile([128, NBLK * CH * SW], mybir.dt.int16)
            wsv = cp.tile([128, NBLK * CH * W], b16)
            cloc = cp.tile([128, NBLK * CH * W], f32)
            nc.scalar.dma_start(out=gidx[:], in_=gidx_d[:])
            nc.scalar.dma_start(out=wsv[:], in_=ws_d[:])
            nc.scalar.dma_start(out=cloc[:], in_=cloc_d[:])

            dinv_all = cp.tile([128, NT], f32)
            dinv_loc = cp.tile([BLK, NBLK], f32)
            ve_all = cp.tile([BLK, NBLK * D], f32)
            b_rep = cp.tile([BLK, D], f32)
            w_all = cp.tile([128, NT * L], f32)
            wl_all = cp.tile([BLK, NBLK * L], f32)
            eT0 = cp.tile([128, N], f32)
            eT1 = cp.tile([128, N], f32)
            g_all = cp.tile([128, NT * D], b16)
            nc.scalar.dma_start(out=w_all[:], in_=wpad_d[:])
            nc.scalar.dma_start(out=wl_all[:], in_=wploc_d[:])
            nc.scalar.dma_start(out=eT0[:], in_=embT_d[0:128, :])
            nc.scalar.dma_start(out=eT1[:], in_=embT_d[128:256, :])

            # ---- phase A: degrees -> dinv ----
            with (
                tc.tile_pool(name="pA", bufs=4) as pA,
                tc.tile_pool(name="ppA", bufs=2, space="PSUM") as ppA,
            ):
                for j in range(NT):
                    deg = pA.tile([128, 1], f32, tag="deg")
                    nc.vector.reduce_sum(
                        out=deg[:],
                        in_=w_all[:, j * L : (j + 1) * L],
                        axis=mybir.AxisListType.X,
                    )
                    rec = pA.tile([128, 1], f32, tag="rec")
                    nc.vector.reciprocal(rec[:], deg[:])
                    nc.scalar.sqrt(dinv_all[:, j : j + 1], rec[:])
                for blk in range(NBLK):
                    deg = pA.tile([BLK, 1], f32, tag="degl")
                    nc.vector.reduce_sum(
                        out=deg[:],
                        in_=wl_all[:, blk * L : (blk + 1) * L],
                        axis=mybir.AxisListType.X,
                    )
                    rec = pA.tile([BLK, 1], f32, tag="recl")
                    nc.vector.reciprocal(rec[:], deg[:])
                    nc.scalar.sqrt(dinv_loc[:, blk : blk + 1], rec[:])

                # ---- phase B: g = dinv * (emb @ W_gcn) -> one DRAM write ----
                nc.vector.memset(g_all[:, (NT - 1) * D :], 0.0)
                for j in range(NT):
                    cols = min(128, N - j * 128)
                    hg = ppA.tile([128, D], f32, space="PSUM", tag="hg")
                    nc.tensor.matmul(
                        hg[:cols, :],
                        lhsT=eT0[:, j * 128 : j * 128 + cols],
                        rhs=wg0[:],
                        start=True,
                        stop=False,
                    )
                    nc.tensor.matmul(
                        hg[:cols, :],
                        lhsT=eT1[:, j * 128 : j * 128 + cols],
                        rhs=wg1[:],
                        start=False,
                        stop=True,
                    )
                    nc.vector.tensor_scalar_mul(
                        g_all[:cols, j * D : (j + 1) * D],
                        hg[:cols, :],
                        dinv_all[:cols, j : j + 1],
                    )
                nc.sync.dma_start(out=g_d[:].rearrange("(p a) d -> p a d", p=128),
                                  in_=g_all[:].rearrange("p (a d) -> p a d", d=D))
                # b_rep = ones(125,1) @ b_gcn(1,256)
                br = ppA.tile([BLK, D], f32, space="PSUM", tag="hg")
                nc.tensor.matmul(
                    br[:], lhsT=ones_row[0:1, :], rhs=bg_row[0:1, :], start=True, stop=True
                )
                nc.vector.tensor_copy(b_rep[:], br[:])

            # ---- phase C: per block, GCN scatter-matmul then main output ----
            with (
                tc.tile_pool(name="pC", bufs=4) as pC,
                tc.tile_pool(name="xload", bufs=2) as xload,
                tc.tile_pool(name="vps", bufs=2, space="PSUM") as vps,
                tc.tile_pool(name="mps", bufs=2, space="PSUM") as mps,
                tc.tile_pool(name="outp", bufs=3) as outp,
            ):
                qi = 0
                for blk in range(NBLK):
                    vp = vps.tile([BLK, D], f32, space="PSUM", tag="vp")
                    for ch in range(CH):
                        c0 = (blk * CH + ch) * W
                        ci = (blk * CH + ch) * SW
                        M = pC.tile([128, W * D], b16, tag="M")
                        nc.gpsimd.dma_gather(
                            out_ap=M[:].rearrange("p (c d) -> p c d", d=D),
                            in_ap=g_d[:],
                            idxs_ap=gidx[:, ci : ci + SW],
                            num_idxs=CHUNK,
                            num_idxs_reg=CHUNK,
                            elem_size=D,
                            single_packet=False,
                            queue_num=qi % 4,
                        )
                        qi += 1
                        # S = (iota == cloc) * ws ; cloc/ws broadcast along BLK
                        S0 = pC.tile([128, W * BLK], b16, tag="S")
                        nc.vector.tensor_tensor(
                            out=S0[:],
                            in0=iota_f[:],
                            in1=cloc[:, c0 : c0 + W, None].to_broadcast(
                                [128, W, BLK]
                            ),
                            op=mybir.AluOpType.is_equal,
                        )
                        nc.vector.tensor_tensor(
                            out=S0[:],
                            in0=S0[:],
                            in1=wsv[:, c0 : c0 + W, None].to_broadcast([128, W, BLK]),
                            op=mybir.AluOpType.mult,
                        )
                        for j in range(W):
                            nc.tensor.matmul(
                                vp[:],
                                lhsT=S0[:, j * BLK : (j + 1) * BLK],
                                rhs=M[:, j * D : (j + 1) * D],
                                start=(ch == 0 and j == 0),
                                stop=(ch == CH - 1 and j == W - 1),
                            )
                    ve = ve_all[:, blk * D : (blk + 1) * D]
                    nc.vector.tensor_scalar_mul(ve, vp[:], dinv_loc[:, blk : blk + 1])
                    nc.vector.tensor_add(ve, ve, b_rep[:])
                    ve3 = ve.rearrange("p d -> p () d").to_broadcast([BLK, T // 2, D])

                    x38t = xload.tile([KX, B * BLK], f32, tag="x")
                    nc.scalar.dma_start(out=x38t[:], in_=x38_d[blk])
                    for b in range(B):
                        lhsT = x38t[:, b * BLK : (b + 1) * BLK]
                        osb = outp.tile([BLK, T * D], f32, tag="osb")
                        for half in range(2):
                            mp = mps.tile([BLK, 3 * 512], f32, space="PSUM", tag="mp")
                            for i in range(3):
                                tp = half * 3 + i  # t-pair index
                                nc.tensor.matmul(
                                    mp[:, i * 512 : (i + 1) * 512],
                                    lhsT=lhsT,
                                    rhs=rhs38[:, tp * 512 : (tp + 1) * 512],
                                    start=True,
                                    stop=True,
                                )
                            nc.vector.tensor_tensor(
                                out=osb[:, half * 1536 : (half + 1) * 1536].rearrange(
                                    "p (t d) -> p t d", d=D
                                ),
                                in0=mp[:].rearrange("p (t d) -> p t d", d=D),
                                in1=ve3,
                                op=mybir.AluOpType.add,
                            )
                        nc.sync.dma_start(
                            out=out_d[b, blk * BLK : (blk + 1) * BLK].rearrange(
                                "p t d -> p (t d)"
                            ),
                            in_=osb[:],
                        )

    nc.finalize()  # run bacc passes (reg alloc, TRN2 sync-wait splitting)
    _KERNEL_CACHE[key] = nc
    return nc


LAST_RESULTS = None  # BassKernelResults of the most recent run (for profiling)


def kernel(x, x_mark, edge_index, weights, W_lin, b_lin, emb_table, W_gcn, b_gcn):
    global LAST_RESULTS
    per_core, wpad, rhs38, CH, L = _prep(x, edge_index, weights, W_lin, b_lin)
    nc = _build_kernel(CH, L)
    embT = np.ascontiguousarray(np.asarray(emb_table, dtype=np.float32).T)
    shared = {
        "wpad": wpad,
        "embT": embT,
        "W_gcn": np.asarray(W_gcn, dtype=np.float32),
        "b_gcn": np.asarray(b_gcn, dtype=np.float32).reshape(1, D),
        "rhs38": rhs38,
    }
    in_maps = [{**shared, **pc} for pc in per_core]
    res = run_bass_kernel_spmd(nc, in_maps, list(range(NCORES)))
    LAST_RESULTS = res
    shards = [res.results[k]["out"] for k in range(NCORES)]
    return np.concatenate(shards, axis=1)
